# revision 23
# baseline (speedup 1.0000x reference)
"""DDPM scheduler kernel for Trainium2 (Bass/Tile), 8-core data parallel.

Computes out = exp(clog[clip(round(t), 0, 1000)]) for t in [0, 1000],
where clog is the cumulative-log-alpha table of the classical DDPM
beta schedule (beta0=1e-4, beta1T/T=0.02, T=1000).

Instead of a 1001-entry table gather (slow on TRN2), we evaluate a
cubic polynomial fit of clog(n) factored into
    P3(u) = SE2 * [(u+H)^2 + O] * (n + F) + BE,   u = n/1024, n = rint(t)
(max |fit err| 9.7e-6 in log domain, at the fp32 table's own ~1.1e-5
noise floor vs the exact curve; F is the far real root scaled by -1024
and rounded to an exact integer so V = n + F is exact in fp32).

Per-chunk engine schedule (Tile framework handles all semaphores):
    DVE : rint (magic-number round-to-nearest-even), Z = Y + O, W = Z*V
    ACT : Y = Square(n/1024 + H), out = Exp(SE2*W + BE)
    V = n + F runs on DVE (variant A) or ACT-Copy (variant B); chunks
    alternate variants in a ratio that equalizes DVE and ACT busy time,
    leaving HBM bandwidth as the only saturated resource.
"""

import numpy as np

import concourse.bacc as bacc
import concourse.mybir as mybir
from concourse.bass_utils import run_bass_kernel_spmd
from concourse.tile import TileContext

N_CORES = 8
TOTAL = 16777216
PER_CORE = TOTAL // N_CORES  # 2097152
P = 128

# fp32 constants (derived offline from the exact fp64 table; see docstring)
MAGIC = 12582912.0  # 1.5 * 2^23: (t + MAGIC) - MAGIC == rint(t) for 0 <= t < 2^22
SCALE = float(np.float32(2.0**-10))
H = float(np.float32(0.0041867206))
O = float(np.float32(0.06839018))
F = 147578.0  # integer: V = rint(t) + F is exact in fp32 (< 2^24)
SE2 = float(np.float32(-7.076394e-05))
BE = float(np.float32(0.7144051))


def build_nc(per_core: int = PER_CORE, plan: list[tuple[int, str]] | None = None):
    # (width, variant) per chunk. Ramped widths: small first chunk so compute
    # starts early, small last chunk so the final store is short. Variant 'A'
    # puts the V = n + F shift on DVE, 'B' on ACT (Copy); the A:B width ratio
    # (~5120:11264) equalizes DVE and ACT busy time.
    if plan is None:
        plan = [
            (512, "A"), (2048, "B"), (1536, "A"), (3072, "B"),
            (1536, "A"), (3072, "B"), (1024, "A"), (2560, "B"),
            (512, "A"), (512, "B"),
        ]
    chunks = [w for w, _ in plan]
    assert sum(chunks) * P == per_core
    pad_f = max(chunks)

    # Bacc (not raw Bass): its finalize() runs generate_event_semaphores(),
    # which splits multi-sem waits into InstEventSemaphore chains -- TRN2
    # allows at most 1 sync-wait per compute instruction.
    nc = bacc.Bacc()
    t_in = nc.dram_tensor("t", [per_core], mybir.dt.float32, kind="ExternalInput")
    y_out = nc.dram_tensor("y", [per_core], mybir.dt.float32, kind="ExternalOutput")

    # Per-chunk views: each chunk is a fully CONTIGUOUS DRAM block (strided
    # per-partition layouts measured ~35% lower HBM bandwidth). The element
    # permutation is mirrored exactly on the output, so any consistent
    # mapping is correct for this purely elementwise kernel.
    def chunk_view(dram, base_elems, cw):
        return dram[base_elems : base_elems + P * cw].rearrange(
            "(p f) -> p f", p=P
        )

    AF = mybir.ActivationFunctionType
    OP = mybir.AluOpType
    f32 = mybir.dt.float32

    with TileContext(nc) as tc:
        with (
            tc.tile_pool(name="const", bufs=1) as const_pool,
            tc.tile_pool(name="in", bufs=4) as in_pool,
            tc.tile_pool(name="io", bufs=3) as io_pool,
            tc.tile_pool(name="wk", bufs=3) as wk_pool,
            tc.tile_pool(name="vv", bufs=2) as vv_pool,
            tc.tile_pool(name="st", bufs=3) as st_pool,
        ):
            bh = const_pool.tile([P, 1], f32, tag="bh")
            nc.gpsimd.memset(bh[:], H)
            bbe = const_pool.tile([P, 1], f32, tag="bbe")
            nc.gpsimd.memset(bbe[:], BE)
            off = 0
            for ci, (cw, variant) in enumerate(plan):
                pad = [P, pad_f]
                tt = in_pool.tile([P, cw], f32, tag="t", padded_shape=pad)
                nc.sync.dma_start(tt[:], chunk_view(t_in, off * P, cw))
                # n = rint(t), exact round-half-to-even (matches jnp.round)
                nc.vector.tensor_scalar(
                    tt[:], tt[:], MAGIC, MAGIC, OP.add, OP.subtract
                )
                # Y = (u + H)^2, u = n/1024 via ACT's fused scale
                y1 = wk_pool.tile([P, cw], f32, tag="y1", padded_shape=pad)
                nc.scalar.activation(y1[:], tt[:], AF.Square, bias=bh[:], scale=SCALE)
                # Z = Y + O (in place; DVE in-place is safe, ACT in-place is NOT)
                nc.vector.tensor_scalar(y1[:], y1[:], O, None, OP.add)
                # V = n + F  (exact: F integer, result < 2^24)
                if variant == "A":
                    # in place on tt (after the Square read); DVE
                    nc.vector.tensor_scalar(tt[:], tt[:], F, None, OP.add)
                    vv = tt
                else:
                    # ACT Copy(scale*x + bias); float bias allowed for Copy
                    vv = vv_pool.tile([P, cw], f32, tag="v", padded_shape=pad)
                    nc.scalar.activation(vv[:], tt[:], AF.Copy, bias=F, scale=1.0)
                # W = Z * V
                y3 = io_pool.tile([P, cw], f32, tag="y3", padded_shape=pad)
                nc.vector.tensor_tensor(y3[:], y1[:], vv[:], OP.mult)
                # out = exp(SE2*W + BE); NOT in place (ACT in-place = garbage)
                yo = st_pool.tile([P, cw], f32, tag="yo", padded_shape=pad)
                nc.scalar.activation(yo[:], y3[:], AF.Exp, bias=bbe[:], scale=SE2)
                # Stores ride the ACT HWDGE ring (separate FIFO from loads on
                # the SP ring); late stores alternate onto the SP ring once
                # the loads have drained, halving tail receipt serialization.
                late = ci >= len(plan) - 3
                st_engine = nc.sync if (late and ci % 2 == 1) else nc.scalar
                st_engine.dma_start(chunk_view(y_out, off * P, cw), yo[:])
                off += cw
    # Bacc.finalize() runs compile() (reg alloc, event-sem legalization);
    # run_bass_via_pjrt serializes nc as-is and needs this done.
    nc.finalize()
    return nc


_nc_cache = None


def kernel(t: np.ndarray) -> np.ndarray:
    global _nc_cache
    assert t.shape == (TOTAL,) and t.dtype == np.float32
    if _nc_cache is None:
        _nc_cache = build_nc()
    nc = _nc_cache
    shards = np.ascontiguousarray(t.reshape(N_CORES, PER_CORE))
    in_maps = [{"t": shards[i]} for i in range(N_CORES)]
    res = run_bass_kernel_spmd(nc, in_maps, core_ids=list(range(N_CORES)))
    return np.concatenate([r["y"] for r in res.results])


# revision 25
# speedup vs baseline: 1.0797x; 1.0797x over previous
"""DDPM scheduler kernel for Trainium2 (Bass/Tile), 8-core data parallel.

Computes out = exp(clog[clip(round(t), 0, 1000)]) for t in [0, 1000],
where clog is the cumulative-log-alpha table of the classical DDPM
beta schedule (beta0=1e-4, beta1T/T=0.02, T=1000).

Instead of a 1001-entry table gather (slow on TRN2), we evaluate a
cubic polynomial fit of clog(n) factored into
    P3(u) = SE2 * [(u+H)^2 + O] * (n + F) + BE,   u = n/1024, n = rint(t)
(max |fit err| 9.7e-6 in log domain, at the fp32 table's own ~1.1e-5
noise floor vs the exact curve; F is the far real root scaled by -1024
and rounded to an exact integer so V = n + F is exact in fp32).

Per-chunk engine schedule (Tile framework handles all semaphores):
    DVE : rint (magic-number round-to-nearest-even), Z = Y + O, W = Z*V
    ACT : Y = Square(n/1024 + H), out = Exp(SE2*W + BE)
    V = n + F runs on DVE (variant A) or ACT-Copy (variant B); chunks
    alternate variants in a ratio that equalizes DVE and ACT busy time,
    leaving HBM bandwidth as the only saturated resource.
"""

import numpy as np

import concourse.bacc as bacc
import concourse.mybir as mybir
from concourse.bass_utils import run_bass_kernel_spmd
from concourse.tile import TileContext

N_CORES = 8
TOTAL = 16777216
PER_CORE = TOTAL // N_CORES  # 2097152
P = 128

# fp32 constants (derived offline from the exact fp64 table; see docstring)
MAGIC = 12582912.0  # 1.5 * 2^23: (t + MAGIC) - MAGIC == rint(t) for 0 <= t < 2^22
SCALE = float(np.float32(2.0**-10))
H = float(np.float32(0.0041867206))
O = float(np.float32(0.06839018))
F = 147578.0  # integer: V = rint(t) + F is exact in fp32 (< 2^24)
SE2 = float(np.float32(-7.076394e-05))
BE = float(np.float32(0.7144051))


def build_nc(
    per_core: int = PER_CORE,
    plan: list[tuple[int, str]] | None = None,
    regions: list[int] | None = None,
):
    # plan: (width, variant) per compute chunk, strict A/B alternation.
    # Variant 'A' puts the V = n + F shift on DVE, 'B' on ACT (Copy); the
    # A:B width ratio (5120:11264) equalizes DVE and ACT busy time.
    # regions: column-range boundaries used for BOTH the bulk loads and the
    # bulk stores. Input stays resident in one 64KB/partition SBUF megatile
    # (which also serves as the store staging buffer -- EXP writes back onto
    # the input columns it consumed). This removes all per-chunk DMA
    # completion receipts and ring contention from the steady state.
    if plan is None:
        plan = [
            (512, "A"), (2048, "B"), (1536, "A"), (3072, "B"),
            (1536, "A"), (3072, "B"), (1536, "A"), (3072, "B"),
        ]
    if regions is None:
        regions = [4096, 8704, 13312, 16384]
    chunks = [w for w, _ in plan]
    assert sum(chunks) * P == per_core
    pad_f = max(chunks)
    width = per_core // P
    assert regions[-1] == width

    # Bacc (not raw Bass): its finalize() runs generate_event_semaphores(),
    # which splits multi-sem waits into InstEventSemaphore chains -- TRN2
    # allows at most 1 sync-wait per compute instruction.
    nc = bacc.Bacc()
    t_in = nc.dram_tensor("t", [per_core], mybir.dt.float32, kind="ExternalInput")
    y_out = nc.dram_tensor("y", [per_core], mybir.dt.float32, kind="ExternalOutput")

    # Region [a, b) maps DRAM[a*P + p*(b-a) + j] <-> sbuf data[p, a+j]:
    # each region is one fully CONTIGUOUS DRAM block (strided per-partition
    # layouts measured ~35% lower HBM bandwidth). Loads and stores use the
    # SAME regions, so every element's output lands at its input's index.
    def region_view(dram, a, b):
        return dram[a * P : b * P].rearrange("(p f) -> p f", p=P)

    AF = mybir.ActivationFunctionType
    OP = mybir.AluOpType
    f32 = mybir.dt.float32

    with TileContext(nc) as tc:
        with (
            tc.tile_pool(name="const", bufs=1) as const_pool,
            tc.tile_pool(name="big", bufs=1) as big_pool,
            tc.tile_pool(name="wk", bufs=3) as wk_pool,
            tc.tile_pool(name="vv", bufs=2) as vv_pool,
        ):
            bh = const_pool.tile([P, 1], f32, tag="bh")
            nc.gpsimd.memset(bh[:], H)
            bbe = const_pool.tile([P, 1], f32, tag="bbe")
            nc.gpsimd.memset(bbe[:], BE)

            data = big_pool.tile([P, width], f32, tag="data")
            # bulk loads, back-to-back on the SP ring
            ra = 0
            for rb in regions:
                nc.sync.dma_start(data[:, ra:rb], region_view(t_in, ra, rb))
                ra = rb

            pad = [P, pad_f]
            off = 0
            next_region = 0
            for ci, (cw, variant) in enumerate(plan):
                sl = data[:, off : off + cw]
                # n = rint(t), exact round-half-to-even (matches jnp.round)
                nc.vector.tensor_scalar(sl, sl, MAGIC, MAGIC, OP.add, OP.subtract)
                # Y = (u + H)^2, u = n/1024 via ACT's fused scale
                y1 = wk_pool.tile([P, cw], f32, tag="y1", padded_shape=pad)
                nc.scalar.activation(y1[:], sl, AF.Square, bias=bh[:], scale=SCALE)
                # Z = Y + O (in place; DVE in-place is safe, ACT in-place is NOT)
                nc.vector.tensor_scalar(y1[:], y1[:], O, None, OP.add)
                # V = n + F  (exact: F integer, result < 2^24)
                if variant == "A":
                    nc.vector.tensor_scalar(sl, sl, F, None, OP.add)  # in place
                    vv = sl
                else:
                    # ACT Copy(scale*x + bias); float bias allowed for Copy
                    vt = vv_pool.tile([P, cw], f32, tag="v", padded_shape=pad)
                    nc.scalar.activation(vt[:], sl, AF.Copy, bias=F, scale=1.0)
                    vv = vt[:]
                # W = Z * V
                y3 = wk_pool.tile([P, cw], f32, tag="y3", padded_shape=pad)
                nc.vector.tensor_tensor(y3[:], y1[:], vv, OP.mult)
                # out = exp(SE2*W + BE), written back onto the megatile's
                # consumed input columns (NOT in place on ACT: different src)
                nc.scalar.activation(sl, y3[:], AF.Exp, bias=bbe[:], scale=SE2)
                off += cw
                # store any region fully covered by completed chunks; the
                # last region rides the (idle) SP ring, others the ACT ring
                while next_region < len(regions) and regions[next_region] <= off:
                    a = regions[next_region - 1] if next_region else 0
                    b = regions[next_region]
                    eng = nc.sync if next_region == len(regions) - 1 else nc.scalar
                    eng.dma_start(region_view(y_out, a, b), data[:, a:b])
                    next_region += 1
    # Bacc.finalize() runs compile() (reg alloc, event-sem legalization);
    # run_bass_via_pjrt serializes nc as-is and needs this done.
    nc.finalize()
    return nc


_nc_cache = None


def kernel(t: np.ndarray) -> np.ndarray:
    global _nc_cache
    assert t.shape == (TOTAL,) and t.dtype == np.float32
    if _nc_cache is None:
        _nc_cache = build_nc()
    nc = _nc_cache
    shards = np.ascontiguousarray(t.reshape(N_CORES, PER_CORE))
    in_maps = [{"t": shards[i]} for i in range(N_CORES)]
    res = run_bass_kernel_spmd(nc, in_maps, core_ids=list(range(N_CORES)))
    return np.concatenate([r["y"] for r in res.results])
